# revision 13
# baseline (speedup 1.0000x reference)
"""Trainium2 Bass kernel for GroundTruthBasedPriorNetwork.

Per-node tiny MLP over a banded DAG, batched over 131072 samples:
    x[b, n, p]  = gt_labels[b, parent_idx[n, p]]          (N=64 nodes, P=8)
    h[b, n, :]  = tanh(W1[n] @ x[b, n, :] + b1[n])        (HID=16)
    mus[b, n]   = W2[n] . h[b, n, :] + b2[n]
    logvars     = zeros

Pure data parallel over 8 NeuronCores (batch split 8x16384).  ScalarE
(tanh over 16.8M elems/core; saturated ACT pays ~790 cycles of access
setup per instruction, so 64 x (128,2048) tiles ~= 151us) is the
roofline engine; everything else is shaped to hide beneath it at the
cold 1.2 GHz PE clock.

The banded DAG means hidden block t (128 dims = nodes 8t..8t+8) only
reads input rows [8t-8, 8t+7) plus a bias row: K=16.  Four blocks are
row-tiled into PE row-groups 0/32/64/96 and run concurrently; the host
prepares band panels xb0/xb1 (quads t=0-3 / t=4-7, 128 partitions with
a ones row per 32-group) so one quad fills a (128, 2048) PSUM tile.
PSUM is exactly 2 quad tiles (2x4 banks, double-buffered).

Layer 2 per block needs only a (128, 8) stationary; four blocks are
col-tiled into col-groups 0/32/64/96, writing partition strips
32j..32j+8 of bank 0 of the SAME l1 quad-tile its Tanh just consumed.
One full-width DVE tensor_scalar_add(b2) evacuates the strips
(inactive lanes carry junk that the host discards).  Bank 0 of each
quad is refilled last (j order 1,2,3,0) so the evacuation overlaps the
other three matmuls.

All PE/ACT/DVE instructions are chained with order-only dependencies
(add_dep_helper) pinning the software-pipelined emission order; the
Tile scheduler's cost model otherwise reorders the FIFO and causes
head-of-line blocking on the evacuate->refill edge.

Outputs leave as two (128, BC) bf16 panels (junk lanes included);
the host extracts node strips and casts.
"""

import os

import numpy as np

NUM_NODES = 64
MAX_P = 8
HID = 16
HFULL = NUM_NODES * HID  # 1024
BATCH = 131072
NCORES = 8
BC = BATCH // NCORES  # 16384 per core
SLAB = 512
NSLAB = BC // SLAB  # 32
QW = 4 * SLAB  # 2048: quad tile width
OC = 2048  # output DMA chunk width
QUADS = ((0, 1, 2, 3), (4, 5, 6, 7))
ICHUNKS = [(0, 1024), (1024, 3072), (4096, 4096), (8192, 4096),
           (12288, 4096)]

_COMPILED = {}


def _bf16(a):
    import ml_dtypes

    return np.asarray(a, np.float32).astype(ml_dtypes.bfloat16)


def _band_lo(t):
    return max(0, 8 * t - 8)


def _build_weights(W1, b1, W2, b2, parent_idx):
    """Host-side preprocessing of the tiny per-node weights."""
    W1 = np.asarray(W1, np.float32)
    b1 = np.asarray(b1, np.float32)
    W2 = np.asarray(W2, np.float32)
    b2 = np.asarray(b2, np.float32)
    parent_idx = np.asarray(parent_idx)

    # W1_full[j, 16n+h] = sum_p [parent_idx[n,p]==j] * W1[n,h,p]
    w1_full = np.zeros((NUM_NODES, HFULL), np.float32)
    for n in range(NUM_NODES):
        for p in range(MAX_P):
            j = int(parent_idx[n, p])
            w1_full[j, 16 * n : 16 * n + 16] += W1[n, :, p]

    # Row-tiled L1 stationaries: w1b[32j+i, 128q+c] = block t=4q+j's
    # weight for band row i (i=15 -> bias b1).
    w1b = np.zeros((128, 2 * 128), np.float32)
    for q, quad in enumerate(QUADS):
        for j, t in enumerate(quad):
            lo = _band_lo(t)
            nrow = 8 * t + 7 - lo if t > 0 else 7
            w1b[32 * j : 32 * j + nrow, 128 * q : 128 * (q + 1)] = \
                w1_full[lo : lo + nrow, 128 * t : 128 * (t + 1)]
            w1b[32 * j + 15, 128 * q : 128 * (q + 1)] = b1.reshape(HFULL)[
                128 * t : 128 * (t + 1)
            ]

    # Col-tiled L2 stationaries: w2c[p, 8t+k] = W2[8t+k, (128t+p)%16]
    # where (128t+p)//16 == 8t+k, else 0.
    w2c = np.zeros((128, NUM_NODES), np.float32)
    for t in range(8):
        for p in range(128):
            hf = 128 * t + p
            n = hf // HID
            w2c[p, n] = W2[n, hf % HID]

    wts = np.zeros((128, 2 * 128 + NUM_NODES), np.float32)
    wts[:, : 2 * 128] = w1b
    wts[:, 2 * 128 :] = w2c

    # b2 packed into evacuation strip layout: col q, partition 32j+i.
    b2r = np.zeros((128, 2), np.float32)
    for q, quad in enumerate(QUADS):
        for j, t in enumerate(quad):
            b2r[32 * j : 32 * j + 8, q] = b2[8 * t : 8 * t + 8]
    return _bf16(wts), np.ascontiguousarray(b2r)


def _build_bands(xc):
    """xc: (BC, 64) fp32 one core's batch. Returns 2 quad band panels."""
    xt = xc.T  # (64, BC)
    outs = []
    for quad in QUADS:
        xb = np.zeros((128, BC), np.float32)
        for j, t in enumerate(quad):
            lo = _band_lo(t)
            nrow = 8 * t + 7 - lo if t > 0 else 7
            xb[32 * j : 32 * j + nrow] = xt[lo : lo + nrow]
            xb[32 * j + 15] = 1.0
        outs.append(_bf16(xb))
    return outs


def _build_nc():
    import concourse.bacc as bacc
    import concourse.mybir as mybir
    import concourse.tile as tile
    from concourse.tile import add_dep_helper
    from contextlib import ExitStack

    f32 = mybir.dt.float32
    bf16 = mybir.dt.bfloat16

    nc = bacc.Bacc("TRN2", target_bir_lowering=False, debug=False,
                   num_devices=NCORES)

    CW = 2 * 128 + NUM_NODES  # 320
    xb_d = [
        nc.dram_tensor(f"xb{q}", [128, BC], bf16, kind="ExternalInput")
        for q in range(2)
    ]
    wts_d = nc.dram_tensor("wts", [128, CW], bf16, kind="ExternalInput")
    b2_d = nc.dram_tensor("b2", [128, 2], f32, kind="ExternalInput")
    out_d = [
        nc.dram_tensor(f"out{q}", [128, BC], bf16, kind="ExternalOutput")
        for q in range(2)
    ]

    last = {}  # per-engine previous instruction, for order pinning

    def pin(key, bi):
        if key in last:
            add_dep_helper(bi.ins, last[key].ins, sync=False,
                           reason="pipeline order")
        last[key] = bi
        return bi

    with tile.TileContext(nc) as tc, ExitStack() as ctx:
        consts = ctx.enter_context(tc.tile_pool(name="consts", bufs=1))
        xb_pool = ctx.enter_context(tc.tile_pool(name="xb", bufs=1))
        out_pool = ctx.enter_context(tc.tile_pool(name="outp", bufs=2))
        h_pool = ctx.enter_context(tc.tile_pool(name="h", bufs=6))
        l1_pool = ctx.enter_context(tc.tile_pool(name="l1", bufs=2, space="PSUM"))

        wts_sb = consts.tile([128, CW], bf16, tag="wts")
        b2_sb = consts.tile([128, 2], f32, tag="b2")
        dummy = consts.tile([128, 8], f32, tag="dummy")
        dummy2 = consts.tile([128, 8], bf16, tag="dummy2")
        nc.sync.dma_start(wts_sb[:], wts_d.ap())
        nc.sync.dma_start(b2_sb[:], b2_d.ap())
        w1b_sb = wts_sb[:, : 2 * 128]
        w2c_sb = wts_sb[:, 2 * 128 :]

        # Pre-trigger the ACT tanh table load (~2.7us) while DMAs run.
        nc.vector.memset(dummy[:], 0.0)
        nc.scalar.activation(dummy2[:], dummy[:],
                             mybir.ActivationFunctionType.Tanh)

        xb_sb = [
            xb_pool.tile([128, BC], bf16, tag=f"xb{q}", name=f"xb_sb{q}")
            for q in range(2)
        ]
        c0, w = ICHUNKS[0]
        for q in range(2):
            nc.sync.dma_start(xb_sb[q][:, c0 : c0 + w],
                              xb_d[q].ap()[:, c0 : c0 + w])
        tc.strict_bb_all_engine_barrier()
        for c0, w in ICHUNKS[1:]:
            for q in range(2):
                nc.sync.dma_start(xb_sb[q][:, c0 : c0 + w],
                                  xb_d[q].ap()[:, c0 : c0 + w])

        out_tiles = {}
        out_fill = {}

        def out_tile(q, k):
            if (q, k) not in out_tiles:
                out_tiles[(q, k)] = out_pool.tile(
                    [128, OC], bf16, tag=f"oq{q}", name=f"out_q{q}_k{k}"
                )
                out_fill[(q, k)] = 0
            return out_tiles[(q, k)]

        h_live = {}
        l1_live = {}

        def emit_l2(s, q):
            h = h_live.pop((s, q))
            l1 = l1_live.pop((s, q))
            for j, t in enumerate(QUADS[q]):
                pin("pe", nc.tensor.matmul(
                    l1[32 * j : 32 * j + 8, 0:SLAB],
                    w2c_sb[:, 8 * t : 8 * t + 8],
                    h[:, SLAB * j : SLAB * (j + 1)],
                    start=True,
                    stop=True,
                    tile_position=(0, 32 * j),
                    skip_group_check=True,
                ))
            k, oo = divmod(s * SLAB, OC)
            ot = out_tile(q, k)
            # Two half-bank evacuations: the j=0 refill of the next slab
            # is also split in half, so its first half only waits on the
            # first half-evacuation (shortens the critical chain).
            H = SLAB // 2
            for p in range(2):
                pin("dve", nc.vector.tensor_scalar_add(
                    ot[:, oo + p * H : oo + (p + 1) * H],
                    l1[:, p * H : (p + 1) * H],
                    b2_sb[:, q : q + 1],
                ))
            out_fill[(q, k)] += 1
            if out_fill[(q, k)] == OC // SLAB:
                nc.sync.dma_start(
                    out_d[q].ap()[:, k * OC : (k + 1) * OC], ot[:]
                )
                del out_tiles[(q, k)]

        for s in range(NSLAB):
            c = s * SLAB
            for q in range(2):
                if s > 0:
                    emit_l2(s - 1, q)
                l1 = l1_pool.tile([128, QW], f32, tag="l1")
                # Bank 0 (j=0) is refilled last and in two halves: the
                # previous slab's mus strips live there; each half only
                # waits on its own half-evacuation.
                for j in (1, 2, 3):
                    pin("pe", nc.tensor.matmul(
                        l1[:, SLAB * j : SLAB * (j + 1)],
                        w1b_sb[32 * j : 32 * j + 16, 128 * q : 128 * (q + 1)],
                        xb_sb[q][32 * j : 32 * j + 16, c : c + SLAB],
                        start=True,
                        stop=True,
                        tile_position=(32 * j, 0),
                    ))
                H = SLAB // 2
                for p in range(2):
                    pin("pe", nc.tensor.matmul(
                        l1[:, p * H : (p + 1) * H],
                        w1b_sb[0:16, 128 * q : 128 * (q + 1)],
                        xb_sb[q][0:16, c + p * H : c + (p + 1) * H],
                        start=True,
                        stop=True,
                        tile_position=(0, 0),
                        skip_group_check=True,
                    ))
                h = h_pool.tile([128, QW], bf16, tag="h")
                pin("act", nc.scalar.activation(
                    h[:], l1[:], mybir.ActivationFunctionType.Tanh))
                h_live[(s, q)] = h
                l1_live[(s, q)] = l1
        for q in range(2):
            emit_l2(NSLAB - 1, q)

    nc.finalize()
    return nc


def _get_nc():
    if "nc" not in _COMPILED:
        _COMPILED["nc"] = _build_nc()
    return _COMPILED["nc"]


def kernel(gt_labels, W1, b1, W2, b2, parent_idx):
    from concourse.bass_utils import run_bass_kernel_spmd

    gt_labels = np.asarray(gt_labels, np.float32)
    wts, b2r = _build_weights(W1, b1, W2, b2, parent_idx)

    in_maps = []
    for c in range(NCORES):
        xb = _build_bands(gt_labels[c * BC : (c + 1) * BC])
        in_maps.append({"xb0": xb[0], "xb1": xb[1], "wts": wts, "b2": b2r})

    nc = _get_nc()
    trace = bool(int(os.environ.get("KERNEL_TRACE", "0")))
    res = run_bass_kernel_spmd(nc, in_maps, list(range(NCORES)), trace=trace)
    if trace and res.exec_time_ns is not None:
        print(f"HW exec time: {res.exec_time_ns} ns")
        _COMPILED["exec_time_ns"] = res.exec_time_ns

    mus = np.empty((BATCH, NUM_NODES), np.float32)
    for c in range(NCORES):
        rows = []
        for q in range(2):
            panel = np.asarray(res.results[c][f"out{q}"], np.float32)
            for j in range(4):
                rows.append(panel[32 * j : 32 * j + 8])  # nodes 32q+8j..+8
        mus[c * BC : (c + 1) * BC] = np.concatenate(rows, axis=0).T
    mus = mus.reshape(BATCH, NUM_NODES, 1)
    logvars = np.zeros_like(mus)
    return mus, logvars


# revision 15
# speedup vs baseline: 1.0793x; 1.0793x over previous
"""Trainium2 Bass kernel for GroundTruthBasedPriorNetwork.

Per-node tiny MLP over a banded DAG, batched over 131072 samples:
    x[b, n, p]  = gt_labels[b, parent_idx[n, p]]          (N=64 nodes, P=8)
    h[b, n, :]  = tanh(W1[n] @ x[b, n, :] + b1[n])        (HID=16)
    mus[b, n]   = W2[n] . h[b, n, :] + b2[n]
    logvars     = zeros

Pure data parallel over 8 NeuronCores (batch split 8x16384).  ScalarE
(tanh over 16.8M elems/core; saturated ACT pays ~790 cycles of access
setup per instruction, so 64 x (128,2048) tiles ~= 151us) is the
roofline engine; everything else is shaped to hide beneath it at the
cold 1.2 GHz PE clock.

The banded DAG means hidden block t (128 dims = nodes 8t..8t+8) only
reads input rows [8t-8, 8t+7) plus a bias row: K=16.  Four blocks are
row-tiled into PE row-groups 0/32/64/96 and run concurrently; the host
prepares band panels xb0/xb1 (quads t=0-3 / t=4-7, 128 partitions with
a ones row per 32-group) so one quad fills a (128, 2048) PSUM tile.
PSUM is exactly 2 quad tiles (2x4 banks, double-buffered).

Layer 2 per block needs only a (128, 8) stationary; four blocks are
col-tiled into col-groups 0/32/64/96, writing partition strips
32j..32j+8 of bank 0 of the SAME l1 quad-tile its Tanh just consumed.
One full-width DVE tensor_scalar_add(b2) evacuates the strips
(inactive lanes carry junk that the host discards).  Bank 0 of each
quad is refilled last (j order 1,2,3,0) so the evacuation overlaps the
other three matmuls.

All PE/ACT/DVE instructions are chained with order-only dependencies
(add_dep_helper) pinning the software-pipelined emission order; the
Tile scheduler's cost model otherwise reorders the FIFO and causes
head-of-line blocking on the evacuate->refill edge.

Outputs leave as two (128, BC) bf16 panels (junk lanes included);
the host extracts node strips and casts.
"""

import os

import numpy as np

NUM_NODES = 64
MAX_P = 8
HID = 16
HFULL = NUM_NODES * HID  # 1024
BATCH = 131072
NCORES = 8
BC = BATCH // NCORES  # 16384 per core
SLAB = 512
NSLAB = BC // SLAB  # 32
QW = 4 * SLAB  # 2048: quad tile width
OC = 2048  # output DMA chunk width
QUADS = ((0, 1, 2, 3), (4, 5, 6, 7))
ICHUNKS = [(0, 1024), (1024, 3072), (4096, 4096), (8192, 4096),
           (12288, 4096)]

_COMPILED = {}


def _bf16(a):
    import ml_dtypes

    return np.asarray(a, np.float32).astype(ml_dtypes.bfloat16)


def _band_lo(t):
    return max(0, 8 * t - 8)


def _build_weights(W1, b1, W2, b2, parent_idx):
    """Host-side preprocessing of the tiny per-node weights."""
    W1 = np.asarray(W1, np.float32)
    b1 = np.asarray(b1, np.float32)
    W2 = np.asarray(W2, np.float32)
    b2 = np.asarray(b2, np.float32)
    parent_idx = np.asarray(parent_idx)

    # W1_full[j, 16n+h] = sum_p [parent_idx[n,p]==j] * W1[n,h,p]
    w1_full = np.zeros((NUM_NODES, HFULL), np.float32)
    for n in range(NUM_NODES):
        for p in range(MAX_P):
            j = int(parent_idx[n, p])
            w1_full[j, 16 * n : 16 * n + 16] += W1[n, :, p]

    # Row-tiled L1 stationaries: w1b[32j+i, 128q+c] = block t=4q+j's
    # weight for band row i (i=15 -> bias b1).
    w1b = np.zeros((128, 2 * 128), np.float32)
    for q, quad in enumerate(QUADS):
        for j, t in enumerate(quad):
            lo = _band_lo(t)
            nrow = 8 * t + 7 - lo if t > 0 else 7
            w1b[32 * j : 32 * j + nrow, 128 * q : 128 * (q + 1)] = \
                w1_full[lo : lo + nrow, 128 * t : 128 * (t + 1)]
            w1b[32 * j + 15, 128 * q : 128 * (q + 1)] = b1.reshape(HFULL)[
                128 * t : 128 * (t + 1)
            ]

    # Col-tiled L2 stationaries: w2c[p, 8t+k] = W2[8t+k, (128t+p)%16]
    # where (128t+p)//16 == 8t+k, else 0.
    w2c = np.zeros((128, NUM_NODES), np.float32)
    for t in range(8):
        for p in range(128):
            hf = 128 * t + p
            n = hf // HID
            w2c[p, n] = W2[n, hf % HID]

    wts = np.zeros((128, 2 * 128 + NUM_NODES), np.float32)
    wts[:, : 2 * 128] = w1b
    wts[:, 2 * 128 :] = w2c

    # b2 packed into evacuation strip layout: col q, partition 32j+i.
    b2r = np.zeros((128, 2), np.float32)
    for q, quad in enumerate(QUADS):
        for j, t in enumerate(quad):
            b2r[32 * j : 32 * j + 8, q] = b2[8 * t : 8 * t + 8]
    return _bf16(wts), np.ascontiguousarray(b2r)


def _build_bands(xc):
    """xc: (BC, 64) fp32 one core's batch. Returns 2 quad band panels."""
    xt = xc.T  # (64, BC)
    outs = []
    for quad in QUADS:
        xb = np.zeros((128, BC), np.float32)
        for j, t in enumerate(quad):
            lo = _band_lo(t)
            nrow = 8 * t + 7 - lo if t > 0 else 7
            xb[32 * j : 32 * j + nrow] = xt[lo : lo + nrow]
            xb[32 * j + 15] = 1.0
        outs.append(_bf16(xb))
    return outs


def _build_nc():
    import concourse.bacc as bacc
    import concourse.mybir as mybir
    import concourse.tile as tile
    from concourse.tile import add_dep_helper
    from contextlib import ExitStack

    f32 = mybir.dt.float32
    bf16 = mybir.dt.bfloat16

    nc = bacc.Bacc("TRN2", target_bir_lowering=False, debug=False,
                   num_devices=NCORES)

    CW = 2 * 128 + NUM_NODES  # 320
    xb_d = [
        nc.dram_tensor(f"xb{q}", [128, BC], bf16, kind="ExternalInput")
        for q in range(2)
    ]
    wts_d = nc.dram_tensor("wts", [128, CW], bf16, kind="ExternalInput")
    b2_d = nc.dram_tensor("b2", [128, 2], f32, kind="ExternalInput")
    out_d = [
        nc.dram_tensor(f"out{q}", [128, BC], bf16, kind="ExternalOutput")
        for q in range(2)
    ]

    last = {}  # per-engine previous instruction, for order pinning

    def pin(key, bi):
        if key in last:
            add_dep_helper(bi.ins, last[key].ins, sync=False,
                           reason="pipeline order")
        last[key] = bi
        return bi

    with tile.TileContext(nc) as tc, ExitStack() as ctx:
        consts = ctx.enter_context(tc.tile_pool(name="consts", bufs=1))
        xb_pool = ctx.enter_context(tc.tile_pool(name="xb", bufs=1))
        out_pool = ctx.enter_context(tc.tile_pool(name="outp", bufs=2))
        h_pool = ctx.enter_context(tc.tile_pool(name="h", bufs=6))
        l1_pool = ctx.enter_context(tc.tile_pool(name="l1", bufs=2, space="PSUM"))

        wts_sb = consts.tile([128, CW], bf16, tag="wts")
        b2_sb = consts.tile([128, 2], f32, tag="b2")
        dummy = consts.tile([128, 8], f32, tag="dummy")
        dummy2 = consts.tile([128, 8], bf16, tag="dummy2")
        nc.sync.dma_start(wts_sb[:], wts_d.ap())
        nc.sync.dma_start(b2_sb[:], b2_d.ap())
        w1b_sb = wts_sb[:, : 2 * 128]
        w2c_sb = wts_sb[:, 2 * 128 :]

        # Pre-trigger the ACT tanh table load (~2.7us) while DMAs run.
        nc.vector.memset(dummy[:], 0.0)
        nc.scalar.activation(dummy2[:], dummy[:],
                             mybir.ActivationFunctionType.Tanh)

        xb_sb = [
            xb_pool.tile([128, BC], bf16, tag=f"xb{q}", name=f"xb_sb{q}")
            for q in range(2)
        ]
        c0, w = ICHUNKS[0]
        for q in range(2):
            nc.sync.dma_start(xb_sb[q][:, c0 : c0 + w],
                              xb_d[q].ap()[:, c0 : c0 + w])
        tc.strict_bb_all_engine_barrier()
        for c0, w in ICHUNKS[1:]:
            for q in range(2):
                nc.sync.dma_start(xb_sb[q][:, c0 : c0 + w],
                                  xb_d[q].ap()[:, c0 : c0 + w])

        out_tiles = {}
        out_fill = {}

        def out_tile(q, k):
            if (q, k) not in out_tiles:
                out_tiles[(q, k)] = out_pool.tile(
                    [128, OC], bf16, tag=f"oq{q}", name=f"out_q{q}_k{k}"
                )
                out_fill[(q, k)] = 0
            return out_tiles[(q, k)]

        h_live = {}
        l1_live = {}

        def emit_l2(s, q):
            h = h_live.pop((s, q))
            l1 = l1_live.pop((s, q))
            for j, t in enumerate(QUADS[q]):
                pin("pe", nc.tensor.matmul(
                    l1[32 * j : 32 * j + 8, 0:SLAB],
                    w2c_sb[:, 8 * t : 8 * t + 8],
                    h[:, SLAB * j : SLAB * (j + 1)],
                    start=True,
                    stop=True,
                    tile_position=(0, 32 * j),
                    skip_group_check=True,
                ))
            k, oo = divmod(s * SLAB, OC)
            ot = out_tile(q, k)
            pin("dve", nc.vector.tensor_scalar_add(
                ot[:, oo : oo + SLAB], l1[:, 0:SLAB], b2_sb[:, q : q + 1]
            ))
            out_fill[(q, k)] += 1
            if out_fill[(q, k)] == OC // SLAB:
                nc.sync.dma_start(
                    out_d[q].ap()[:, k * OC : (k + 1) * OC], ot[:]
                )
                del out_tiles[(q, k)]

        for s in range(NSLAB):
            c = s * SLAB
            for q in range(2):
                if s > 0:
                    emit_l2(s - 1, q)
                l1 = l1_pool.tile([128, QW], f32, tag="l1")
                # Bank 0 (j=0) is refilled last: the previous slab's mus
                # strips live there and the DVE evacuation overlaps the
                # j=1..3 matmuls.
                for j in (1, 2, 3, 0):
                    pin("pe", nc.tensor.matmul(
                        l1[:, SLAB * j : SLAB * (j + 1)],
                        w1b_sb[32 * j : 32 * j + 16, 128 * q : 128 * (q + 1)],
                        xb_sb[q][32 * j : 32 * j + 16, c : c + SLAB],
                        start=True,
                        stop=True,
                        tile_position=(32 * j, 0),
                    ))
                h = h_pool.tile([128, QW], bf16, tag="h")
                pin("act", nc.scalar.activation(
                    h[:], l1[:], mybir.ActivationFunctionType.Tanh))
                h_live[(s, q)] = h
                l1_live[(s, q)] = l1
        for q in range(2):
            emit_l2(NSLAB - 1, q)

    nc.finalize()
    return nc


def _get_nc():
    if "nc" not in _COMPILED:
        _COMPILED["nc"] = _build_nc()
    return _COMPILED["nc"]


def kernel(gt_labels, W1, b1, W2, b2, parent_idx):
    from concourse.bass_utils import run_bass_kernel_spmd

    gt_labels = np.asarray(gt_labels, np.float32)
    wts, b2r = _build_weights(W1, b1, W2, b2, parent_idx)

    in_maps = []
    for c in range(NCORES):
        xb = _build_bands(gt_labels[c * BC : (c + 1) * BC])
        in_maps.append({"xb0": xb[0], "xb1": xb[1], "wts": wts, "b2": b2r})

    nc = _get_nc()
    trace = bool(int(os.environ.get("KERNEL_TRACE", "0")))
    res = run_bass_kernel_spmd(nc, in_maps, list(range(NCORES)), trace=trace)
    if trace and res.exec_time_ns is not None:
        print(f"HW exec time: {res.exec_time_ns} ns")
        _COMPILED["exec_time_ns"] = res.exec_time_ns

    mus = np.empty((BATCH, NUM_NODES), np.float32)
    for c in range(NCORES):
        rows = []
        for q in range(2):
            panel = np.asarray(res.results[c][f"out{q}"], np.float32)
            for j in range(4):
                rows.append(panel[32 * j : 32 * j + 8])  # nodes 32q+8j..+8
        mus[c * BC : (c + 1) * BC] = np.concatenate(rows, axis=0).T
    mus = mus.reshape(BATCH, NUM_NODES, 1)
    logvars = np.zeros_like(mus)
    return mus, logvars


# revision 17
# speedup vs baseline: 1.0881x; 1.0082x over previous
"""Trainium2 Bass kernel for GroundTruthBasedPriorNetwork.

Per-node tiny MLP over a banded DAG, batched over 131072 samples:
    x[b, n, p]  = gt_labels[b, parent_idx[n, p]]          (N=64 nodes, P=8)
    h[b, n, :]  = tanh(W1[n] @ x[b, n, :] + b1[n])        (HID=16)
    mus[b, n]   = W2[n] . h[b, n, :] + b2[n]
    logvars     = zeros

Pure data parallel over 8 NeuronCores (batch split 8x16384).  ScalarE
(tanh over 16.8M elems/core; saturated ACT pays ~790 cycles of access
setup per instruction, so 64 x (128,2048) tiles ~= 151us) is the
roofline engine; everything else is shaped to hide beneath it at the
cold 1.2 GHz PE clock.

The banded DAG means hidden block t (128 dims = nodes 8t..8t+8) only
reads input rows [8t-8, 8t+7) plus a bias row: K=16.  Four blocks are
row-tiled into PE row-groups 0/32/64/96 and run concurrently; the host
prepares band panels xb0/xb1 (quads t=0-3 / t=4-7, 128 partitions with
a ones row per 32-group) so one quad fills a (128, 2048) PSUM tile.
PSUM is exactly 2 quad tiles (2x4 banks, double-buffered).

Layer 2 per block needs only a (128, 8) stationary; four blocks are
col-tiled into col-groups 0/32/64/96, writing partition strips
32j..32j+8 of bank 0 of the SAME l1 quad-tile its Tanh just consumed.
One full-width DVE tensor_scalar_add(b2) evacuates the strips
(inactive lanes carry junk that the host discards).  Bank 0 of each
quad is refilled last (j order 1,2,3,0) so the evacuation overlaps the
other three matmuls.

All PE/ACT/DVE instructions are chained with order-only dependencies
(add_dep_helper) pinning the software-pipelined emission order; the
Tile scheduler's cost model otherwise reorders the FIFO and causes
head-of-line blocking on the evacuate->refill edge.

Outputs leave as two (128, BC) bf16 panels (junk lanes included);
the host extracts node strips and casts.
"""

import os

import numpy as np

NUM_NODES = 64
MAX_P = 8
HID = 16
HFULL = NUM_NODES * HID  # 1024
BATCH = 131072
NCORES = 8
BC = BATCH // NCORES  # 16384 per core
SLAB = 512
NSLAB = BC // SLAB  # 32
QW = 4 * SLAB  # 2048: quad tile width
OC = 2048  # output DMA chunk width
QUADS = ((0, 1, 2, 3), (4, 5, 6, 7))
ICHUNKS = [(0, 1024), (1024, 3072), (4096, 4096), (8192, 4096),
           (12288, 4096)]

_COMPILED = {}


def _bf16(a):
    import ml_dtypes

    return np.asarray(a, np.float32).astype(ml_dtypes.bfloat16)


def _band_lo(t):
    return max(0, 8 * t - 8)


def _build_weights(W1, b1, W2, b2, parent_idx):
    """Host-side preprocessing of the tiny per-node weights."""
    W1 = np.asarray(W1, np.float32)
    b1 = np.asarray(b1, np.float32)
    W2 = np.asarray(W2, np.float32)
    b2 = np.asarray(b2, np.float32)
    parent_idx = np.asarray(parent_idx)

    # W1_full[j, 16n+h] = sum_p [parent_idx[n,p]==j] * W1[n,h,p]
    w1_full = np.zeros((NUM_NODES, HFULL), np.float32)
    for n in range(NUM_NODES):
        for p in range(MAX_P):
            j = int(parent_idx[n, p])
            w1_full[j, 16 * n : 16 * n + 16] += W1[n, :, p]

    # Row-tiled L1 stationaries: w1b[32j+i, 128q+c] = block t=4q+j's
    # weight for band row i (i=15 -> bias b1).
    w1b = np.zeros((128, 2 * 128), np.float32)
    for q, quad in enumerate(QUADS):
        for j, t in enumerate(quad):
            lo = _band_lo(t)
            nrow = 8 * t + 7 - lo if t > 0 else 7
            w1b[32 * j : 32 * j + nrow, 128 * q : 128 * (q + 1)] = \
                w1_full[lo : lo + nrow, 128 * t : 128 * (t + 1)]
            w1b[32 * j + 15, 128 * q : 128 * (q + 1)] = b1.reshape(HFULL)[
                128 * t : 128 * (t + 1)
            ]

    # Col-tiled L2 stationaries: w2c[p, 8t+k] = W2[8t+k, (128t+p)%16]
    # where (128t+p)//16 == 8t+k, else 0.
    w2c = np.zeros((128, NUM_NODES), np.float32)
    for t in range(8):
        for p in range(128):
            hf = 128 * t + p
            n = hf // HID
            w2c[p, n] = W2[n, hf % HID]

    wts = np.zeros((128, 2 * 128 + NUM_NODES), np.float32)
    wts[:, : 2 * 128] = w1b
    wts[:, 2 * 128 :] = w2c

    # b2 packed into evacuation strip layout: col q, partition 32j+i.
    b2r = np.zeros((128, 2), np.float32)
    for q, quad in enumerate(QUADS):
        for j, t in enumerate(quad):
            b2r[32 * j : 32 * j + 8, q] = b2[8 * t : 8 * t + 8]
    return _bf16(wts), np.ascontiguousarray(b2r)


def _build_bands(xc):
    """xc: (BC, 64) fp32 one core's batch. Returns 2 quad band panels."""
    xt = xc.T  # (64, BC)
    outs = []
    for quad in QUADS:
        xb = np.zeros((128, BC), np.float32)
        for j, t in enumerate(quad):
            lo = _band_lo(t)
            nrow = 8 * t + 7 - lo if t > 0 else 7
            xb[32 * j : 32 * j + nrow] = xt[lo : lo + nrow]
            xb[32 * j + 15] = 1.0
        outs.append(_bf16(xb))
    return outs


def _build_nc():
    import concourse.bacc as bacc
    import concourse.mybir as mybir
    import concourse.tile as tile
    from concourse.tile import add_dep_helper
    from contextlib import ExitStack

    f32 = mybir.dt.float32
    bf16 = mybir.dt.bfloat16

    nc = bacc.Bacc("TRN2", target_bir_lowering=False, debug=False,
                   num_devices=NCORES)

    CW = 2 * 128 + NUM_NODES  # 320
    xb_d = [
        nc.dram_tensor(f"xb{q}", [128, BC], bf16, kind="ExternalInput")
        for q in range(2)
    ]
    wts_d = nc.dram_tensor("wts", [128, CW], bf16, kind="ExternalInput")
    b2_d = nc.dram_tensor("b2", [128, 2], f32, kind="ExternalInput")
    out_d = [
        nc.dram_tensor(f"out{q}", [128, BC], bf16, kind="ExternalOutput")
        for q in range(2)
    ]

    last = {}  # per-engine previous instruction, for order pinning

    def pin(key, bi):
        if key in last:
            add_dep_helper(bi.ins, last[key].ins, sync=False,
                           reason="pipeline order")
        last[key] = bi
        return bi

    with tile.TileContext(nc) as tc, ExitStack() as ctx:
        consts = ctx.enter_context(tc.tile_pool(name="consts", bufs=1))
        xb_pool = ctx.enter_context(tc.tile_pool(name="xb", bufs=1))
        out_pool = ctx.enter_context(tc.tile_pool(name="outp", bufs=2))
        h_pool = ctx.enter_context(tc.tile_pool(name="h", bufs=6))
        l1_pool = ctx.enter_context(tc.tile_pool(name="l1", bufs=2, space="PSUM"))

        wts_sb = consts.tile([128, CW], bf16, tag="wts")
        b2_sb = consts.tile([128, 2], f32, tag="b2")
        dummy = consts.tile([128, 8], f32, tag="dummy")
        dummy2 = consts.tile([128, 8], bf16, tag="dummy2")

        # Pre-trigger the ACT tanh table load (~2.7us) while DMAs run;
        # the memset goes on GpSimd, the earliest-booting engine.
        nc.gpsimd.memset(dummy[:], 0.0)
        nc.scalar.activation(dummy2[:], dummy[:],
                             mybir.ActivationFunctionType.Tanh)

        xb_sb = [
            xb_pool.tile([128, BC], bf16, tag=f"xb{q}", name=f"xb_sb{q}")
            for q in range(2)
        ]
        c0, w = ICHUNKS[0]
        for q in range(2):
            nc.sync.dma_start(xb_sb[q][:, c0 : c0 + w],
                              xb_d[q].ap()[:, c0 : c0 + w])
        nc.sync.dma_start(wts_sb[:], wts_d.ap())
        nc.sync.dma_start(b2_sb[:], b2_d.ap())
        w1b_sb = wts_sb[:, : 2 * 128]
        w2c_sb = wts_sb[:, 2 * 128 :]
        for c0, w in ICHUNKS[1:]:
            for q in range(2):
                nc.sync.dma_start(xb_sb[q][:, c0 : c0 + w],
                                  xb_d[q].ap()[:, c0 : c0 + w])

        out_tiles = {}
        out_fill = {}

        def out_tile(q, k):
            if (q, k) not in out_tiles:
                out_tiles[(q, k)] = out_pool.tile(
                    [128, OC], bf16, tag=f"oq{q}", name=f"out_q{q}_k{k}"
                )
                out_fill[(q, k)] = 0
            return out_tiles[(q, k)]

        h_live = {}
        l1_live = {}

        def emit_l2(s, q):
            h = h_live.pop((s, q))
            l1 = l1_live.pop((s, q))
            for j, t in enumerate(QUADS[q]):
                pin("pe", nc.tensor.matmul(
                    l1[32 * j : 32 * j + 8, 0:SLAB],
                    w2c_sb[:, 8 * t : 8 * t + 8],
                    h[:, SLAB * j : SLAB * (j + 1)],
                    start=True,
                    stop=True,
                    tile_position=(0, 32 * j),
                    skip_group_check=True,
                ))
            k, oo = divmod(s * SLAB, OC)
            ot = out_tile(q, k)
            pin("dve", nc.vector.tensor_scalar_add(
                ot[:, oo : oo + SLAB], l1[:, 0:SLAB], b2_sb[:, q : q + 1]
            ))
            out_fill[(q, k)] += 1
            if out_fill[(q, k)] == OC // SLAB:
                nc.sync.dma_start(
                    out_d[q].ap()[:, k * OC : (k + 1) * OC], ot[:]
                )
                del out_tiles[(q, k)]

        for s in range(NSLAB):
            c = s * SLAB
            for q in range(2):
                if s > 0:
                    emit_l2(s - 1, q)
                l1 = l1_pool.tile([128, QW], f32, tag="l1")
                # Bank 0 (j=0) is refilled last: the previous slab's mus
                # strips live there and the DVE evacuation overlaps the
                # j=1..3 matmuls.
                for j in (1, 2, 3, 0):
                    pin("pe", nc.tensor.matmul(
                        l1[:, SLAB * j : SLAB * (j + 1)],
                        w1b_sb[32 * j : 32 * j + 16, 128 * q : 128 * (q + 1)],
                        xb_sb[q][32 * j : 32 * j + 16, c : c + SLAB],
                        start=True,
                        stop=True,
                        tile_position=(32 * j, 0),
                    ))
                h = h_pool.tile([128, QW], bf16, tag="h")
                pin("act", nc.scalar.activation(
                    h[:], l1[:], mybir.ActivationFunctionType.Tanh))
                h_live[(s, q)] = h
                l1_live[(s, q)] = l1
        for q in range(2):
            emit_l2(NSLAB - 1, q)

    nc.finalize()
    return nc


def _get_nc():
    if "nc" not in _COMPILED:
        _COMPILED["nc"] = _build_nc()
    return _COMPILED["nc"]


def kernel(gt_labels, W1, b1, W2, b2, parent_idx):
    from concourse.bass_utils import run_bass_kernel_spmd

    gt_labels = np.asarray(gt_labels, np.float32)
    wts, b2r = _build_weights(W1, b1, W2, b2, parent_idx)

    in_maps = []
    for c in range(NCORES):
        xb = _build_bands(gt_labels[c * BC : (c + 1) * BC])
        in_maps.append({"xb0": xb[0], "xb1": xb[1], "wts": wts, "b2": b2r})

    nc = _get_nc()
    trace = bool(int(os.environ.get("KERNEL_TRACE", "0")))
    res = run_bass_kernel_spmd(nc, in_maps, list(range(NCORES)), trace=trace)
    if trace and res.exec_time_ns is not None:
        print(f"HW exec time: {res.exec_time_ns} ns")
        _COMPILED["exec_time_ns"] = res.exec_time_ns

    mus = np.empty((BATCH, NUM_NODES), np.float32)
    for c in range(NCORES):
        rows = []
        for q in range(2):
            panel = np.asarray(res.results[c][f"out{q}"], np.float32)
            for j in range(4):
                rows.append(panel[32 * j : 32 * j + 8])  # nodes 32q+8j..+8
        mus[c * BC : (c + 1) * BC] = np.concatenate(rows, axis=0).T
    mus = mus.reshape(BATCH, NUM_NODES, 1)
    logvars = np.zeros_like(mus)
    return mus, logvars


# revision 18
# speedup vs baseline: 1.0964x; 1.0077x over previous
"""Trainium2 Bass kernel for GroundTruthBasedPriorNetwork.

Per-node tiny MLP over a banded DAG, batched over 131072 samples:
    x[b, n, p]  = gt_labels[b, parent_idx[n, p]]          (N=64 nodes, P=8)
    h[b, n, :]  = tanh(W1[n] @ x[b, n, :] + b1[n])        (HID=16)
    mus[b, n]   = W2[n] . h[b, n, :] + b2[n]
    logvars     = zeros

Pure data parallel over 8 NeuronCores (batch split 8x16384).  ScalarE
(tanh over 16.8M elems/core; saturated ACT pays ~790 cycles of access
setup per instruction, so 64 x (128,2048) tiles ~= 151us) is the
roofline engine; everything else is shaped to hide beneath it at the
cold 1.2 GHz PE clock.

The banded DAG means hidden block t (128 dims = nodes 8t..8t+8) only
reads input rows [8t-8, 8t+7) plus a bias row: K=16.  Four blocks are
row-tiled into PE row-groups 0/32/64/96 and run concurrently; the host
prepares band panels xb0/xb1 (quads t=0-3 / t=4-7, 128 partitions with
a ones row per 32-group) so one quad fills a (128, 2048) PSUM tile.
PSUM is exactly 2 quad tiles (2x4 banks, double-buffered).

Layer 2 per block needs only a (128, 8) stationary; four blocks are
col-tiled into col-groups 0/32/64/96, writing partition strips
32j..32j+8 of bank 0 of the SAME l1 quad-tile its Tanh just consumed.
One full-width DVE tensor_scalar_add(b2) evacuates the strips
(inactive lanes carry junk that the host discards).  Bank 0 of each
quad is refilled last (j order 1,2,3,0) so the evacuation overlaps the
other three matmuls.

All PE/ACT/DVE instructions are chained with order-only dependencies
(add_dep_helper) pinning the software-pipelined emission order; the
Tile scheduler's cost model otherwise reorders the FIFO and causes
head-of-line blocking on the evacuate->refill edge.

Outputs leave as two (128, BC) bf16 panels (junk lanes included);
the host extracts node strips and casts.
"""

import os

import numpy as np

NUM_NODES = 64
MAX_P = 8
HID = 16
HFULL = NUM_NODES * HID  # 1024
BATCH = 131072
NCORES = 8
BC = BATCH // NCORES  # 16384 per core
SLAB = 512
NSLAB = BC // SLAB  # 32
QW = 4 * SLAB  # 2048: quad tile width
OC = 2048  # output DMA chunk width
QUADS = ((0, 1, 2, 3), (4, 5, 6, 7))
ICHUNKS = [(0, 1024), (1024, 3072), (4096, 4096), (8192, 4096),
           (12288, 4096)]

_COMPILED = {}


def _bf16(a):
    import ml_dtypes

    return np.asarray(a, np.float32).astype(ml_dtypes.bfloat16)


def _band_lo(t):
    return max(0, 8 * t - 8)


def _build_weights(W1, b1, W2, b2, parent_idx):
    """Host-side preprocessing of the tiny per-node weights."""
    W1 = np.asarray(W1, np.float32)
    b1 = np.asarray(b1, np.float32)
    W2 = np.asarray(W2, np.float32)
    b2 = np.asarray(b2, np.float32)
    parent_idx = np.asarray(parent_idx)

    # W1_full[j, 16n+h] = sum_p [parent_idx[n,p]==j] * W1[n,h,p]
    w1_full = np.zeros((NUM_NODES, HFULL), np.float32)
    for n in range(NUM_NODES):
        for p in range(MAX_P):
            j = int(parent_idx[n, p])
            w1_full[j, 16 * n : 16 * n + 16] += W1[n, :, p]

    # Row-tiled L1 stationaries: w1b[32j+i, 128q+c] = block t=4q+j's
    # weight for band row i (i=15 -> bias b1).
    w1b = np.zeros((128, 2 * 128), np.float32)
    for q, quad in enumerate(QUADS):
        for j, t in enumerate(quad):
            lo = _band_lo(t)
            nrow = 8 * t + 7 - lo if t > 0 else 7
            w1b[32 * j : 32 * j + nrow, 128 * q : 128 * (q + 1)] = \
                w1_full[lo : lo + nrow, 128 * t : 128 * (t + 1)]
            w1b[32 * j + 15, 128 * q : 128 * (q + 1)] = b1.reshape(HFULL)[
                128 * t : 128 * (t + 1)
            ]

    # Col-tiled L2 stationaries: w2c[p, 8t+k] = W2[8t+k, (128t+p)%16]
    # where (128t+p)//16 == 8t+k, else 0.
    w2c = np.zeros((128, NUM_NODES), np.float32)
    for t in range(8):
        for p in range(128):
            hf = 128 * t + p
            n = hf // HID
            w2c[p, n] = W2[n, hf % HID]

    wts = np.zeros((128, 2 * 128 + NUM_NODES), np.float32)
    wts[:, : 2 * 128] = w1b
    wts[:, 2 * 128 :] = w2c

    # b2 packed into evacuation strip layout: col q, partition 32j+i.
    b2r = np.zeros((128, 2), np.float32)
    for q, quad in enumerate(QUADS):
        for j, t in enumerate(quad):
            b2r[32 * j : 32 * j + 8, q] = b2[8 * t : 8 * t + 8]
    return _bf16(wts), np.ascontiguousarray(b2r)


def _build_bands(xc):
    """xc: (BC, 64) fp32 one core's batch. Returns 2 quad band panels."""
    xt = xc.T  # (64, BC)
    outs = []
    for quad in QUADS:
        xb = np.zeros((128, BC), np.float32)
        for j, t in enumerate(quad):
            lo = _band_lo(t)
            nrow = 8 * t + 7 - lo if t > 0 else 7
            xb[32 * j : 32 * j + nrow] = xt[lo : lo + nrow]
            xb[32 * j + 15] = 1.0
        outs.append(_bf16(xb))
    return outs


def _build_nc():
    import concourse.bacc as bacc
    import concourse.mybir as mybir
    import concourse.tile as tile
    from concourse.tile import add_dep_helper
    from contextlib import ExitStack

    f32 = mybir.dt.float32
    bf16 = mybir.dt.bfloat16

    nc = bacc.Bacc("TRN2", target_bir_lowering=False, debug=False,
                   num_devices=NCORES)

    CW = 2 * 128 + NUM_NODES  # 320
    xb_d = [
        nc.dram_tensor(f"xb{q}", [128, BC], bf16, kind="ExternalInput")
        for q in range(2)
    ]
    wts_d = nc.dram_tensor("wts", [128, CW], bf16, kind="ExternalInput")
    b2_d = nc.dram_tensor("b2", [128, 2], f32, kind="ExternalInput")
    out_d = [
        nc.dram_tensor(f"out{q}", [128, BC], bf16, kind="ExternalOutput")
        for q in range(2)
    ]

    last = {}  # per-engine previous instruction, for order pinning

    def pin(key, bi):
        if key in last:
            add_dep_helper(bi.ins, last[key].ins, sync=False,
                           reason="pipeline order")
        last[key] = bi
        return bi

    with tile.TileContext(nc) as tc, ExitStack() as ctx:
        consts = ctx.enter_context(tc.tile_pool(name="consts", bufs=1))
        xb_pool = ctx.enter_context(tc.tile_pool(name="xb", bufs=1))
        out_pool = ctx.enter_context(tc.tile_pool(name="outp", bufs=2))
        h_pool = ctx.enter_context(tc.tile_pool(name="h", bufs=6))
        l1_pool = ctx.enter_context(tc.tile_pool(name="l1", bufs=2, space="PSUM"))

        wts_sb = consts.tile([128, CW], bf16, tag="wts")
        b2_sb = consts.tile([128, 2], f32, tag="b2")
        dummy = consts.tile([128, 8], f32, tag="dummy")
        dummy2 = consts.tile([128, 8], bf16, tag="dummy2")

        # Pre-trigger the ACT tanh table load (~2.7us) while DMAs run;
        # the memset goes on GpSimd, the earliest-booting engine.
        nc.gpsimd.memset(dummy[:], 0.0)
        nc.scalar.activation(dummy2[:], dummy[:],
                             mybir.ActivationFunctionType.Tanh)

        xb_sb = [
            xb_pool.tile([128, BC], bf16, tag=f"xb{q}", name=f"xb_sb{q}")
            for q in range(2)
        ]
        c0, w = ICHUNKS[0]
        for q in range(2):
            nc.sync.dma_start(xb_sb[q][:, c0 : c0 + w],
                              xb_d[q].ap()[:, c0 : c0 + w])
        nc.sync.dma_start(wts_sb[:], wts_d.ap())
        nc.sync.dma_start(b2_sb[:], b2_d.ap())
        w1b_sb = wts_sb[:, : 2 * 128]
        w2c_sb = wts_sb[:, 2 * 128 :]
        for c0, w in ICHUNKS[1:]:
            for q in range(2):
                nc.sync.dma_start(xb_sb[q][:, c0 : c0 + w],
                                  xb_d[q].ap()[:, c0 : c0 + w])

        out_tiles = {}
        out_fill = {}

        def out_tile(q, k):
            if (q, k) not in out_tiles:
                out_tiles[(q, k)] = out_pool.tile(
                    [128, OC], bf16, tag=f"oq{q}", name=f"out_q{q}_k{k}"
                )
                out_fill[(q, k)] = 0
            return out_tiles[(q, k)]

        h_live = {}
        l1_live = {}

        def emit_l2(s, q):
            h = h_live.pop((s, q))
            l1 = l1_live.pop((s, q))
            for j, t in enumerate(QUADS[q]):
                pin("pe", nc.tensor.matmul(
                    l1[32 * j : 32 * j + 8, 0:SLAB],
                    w2c_sb[:, 8 * t : 8 * t + 8],
                    h[:, SLAB * j : SLAB * (j + 1)],
                    start=True,
                    stop=True,
                    tile_position=(0, 32 * j),
                    skip_group_check=True,
                ))
            k, oo = divmod(s * SLAB, OC)
            ot = out_tile(q, k)
            pin("dve", nc.vector.tensor_scalar_add(
                ot[:, oo : oo + SLAB], l1[:, 0:SLAB], b2_sb[:, q : q + 1]
            ))
            out_fill[(q, k)] += 1
            if k == BC // OC - 1:
                # Final chunk: DMA per slab so the kernel tail only waits
                # on one small last transfer.
                lo = (out_fill[(q, k)] - 1) * SLAB
                nc.sync.dma_start(
                    out_d[q].ap()[:, k * OC + lo : k * OC + lo + SLAB],
                    ot[:, lo : lo + SLAB],
                )
                if out_fill[(q, k)] == OC // SLAB:
                    del out_tiles[(q, k)]
            elif out_fill[(q, k)] == OC // SLAB:
                nc.sync.dma_start(
                    out_d[q].ap()[:, k * OC : (k + 1) * OC], ot[:]
                )
                del out_tiles[(q, k)]

        for s in range(NSLAB):
            c = s * SLAB
            for q in range(2):
                if s > 0:
                    emit_l2(s - 1, q)
                l1 = l1_pool.tile([128, QW], f32, tag="l1")
                # Bank 0 (j=0) is refilled last: the previous slab's mus
                # strips live there and the DVE evacuation overlaps the
                # j=1..3 matmuls.
                for j in (1, 2, 3, 0):
                    pin("pe", nc.tensor.matmul(
                        l1[:, SLAB * j : SLAB * (j + 1)],
                        w1b_sb[32 * j : 32 * j + 16, 128 * q : 128 * (q + 1)],
                        xb_sb[q][32 * j : 32 * j + 16, c : c + SLAB],
                        start=True,
                        stop=True,
                        tile_position=(32 * j, 0),
                    ))
                h = h_pool.tile([128, QW], bf16, tag="h")
                pin("act", nc.scalar.activation(
                    h[:], l1[:], mybir.ActivationFunctionType.Tanh))
                h_live[(s, q)] = h
                l1_live[(s, q)] = l1
        for q in range(2):
            emit_l2(NSLAB - 1, q)

    nc.finalize()
    return nc


def _get_nc():
    if "nc" not in _COMPILED:
        _COMPILED["nc"] = _build_nc()
    return _COMPILED["nc"]


def kernel(gt_labels, W1, b1, W2, b2, parent_idx):
    from concourse.bass_utils import run_bass_kernel_spmd

    gt_labels = np.asarray(gt_labels, np.float32)
    wts, b2r = _build_weights(W1, b1, W2, b2, parent_idx)

    in_maps = []
    for c in range(NCORES):
        xb = _build_bands(gt_labels[c * BC : (c + 1) * BC])
        in_maps.append({"xb0": xb[0], "xb1": xb[1], "wts": wts, "b2": b2r})

    nc = _get_nc()
    trace = bool(int(os.environ.get("KERNEL_TRACE", "0")))
    res = run_bass_kernel_spmd(nc, in_maps, list(range(NCORES)), trace=trace)
    if trace and res.exec_time_ns is not None:
        print(f"HW exec time: {res.exec_time_ns} ns")
        _COMPILED["exec_time_ns"] = res.exec_time_ns

    mus = np.empty((BATCH, NUM_NODES), np.float32)
    for c in range(NCORES):
        rows = []
        for q in range(2):
            panel = np.asarray(res.results[c][f"out{q}"], np.float32)
            for j in range(4):
                rows.append(panel[32 * j : 32 * j + 8])  # nodes 32q+8j..+8
        mus[c * BC : (c + 1) * BC] = np.concatenate(rows, axis=0).T
    mus = mus.reshape(BATCH, NUM_NODES, 1)
    logvars = np.zeros_like(mus)
    return mus, logvars
